# revision 1
# baseline (speedup 1.0000x reference)
"""Radial power-spectrum (GroupStat.get_spectrum) Trainium2 kernel.

Math:  out[b,c,r] = sum_{p: idx[p]==r} x[b,c,p]^2 * w[p] / (cnt[r]+eps)

Strategy (8 NeuronCores, data-parallel over batch B=128 -> 16 per core):
  * per core n = 16*8 = 128 rows (b_local, c) -> exactly the 128 SBUF
    partitions; pixels p = 256*129 = 33024 = 258 chunks of 128.
  * fold w[p]/(cnt[idx[p]]+eps) into a single per-pixel scalar wt[p] (host).
  * device pipeline per chunk:
      - DMA big fp32 tiles [128n, F] of x (natural layout)
      - ScalarE: square, cast -> fp16
      - xbar DMA-transpose [128n,128p] -> [128p,128n] fp16
      - DVE: weighted one-hot [128p, 130r] = (iota == idx[p]) * wt[p]
        (single tensor_scalar, op0=is_equal, op1=mult)
      - PE: psum[128n, 130r] += xT.T @ onehot  (258 accumulating matmuls)
  * psum -> SBUF -> DRAM [128, 129] per core; host stacks to [128,8,129].
"""

import math
import os
from contextlib import ExitStack

import numpy as np

from concourse import bass, bacc, mybir
import concourse.tile as tile
from concourse.bass_utils import run_bass_kernel_spmd

B, C, S, XDIM = 128, 8, 256, 129
MAX_R = XDIM  # 129 shells
EPS = 1e-5
NCORES = 8
BLOC = B // NCORES          # 16 batches per core
NROW = BLOC * C             # 128 rows per core -> partition dim
NPIX = S * XDIM             # 33024 pixels
PCHUNK = 128
NCHUNK = NPIX // PCHUNK     # 258 (exact)
RPAD = 130                  # even free dim for DVE 4x mode; col 129 unused

F32 = mybir.dt.float32
F16 = mybir.dt.float16

# transpose path: "xbar" (DMA transpose) or "pe" (TensorE transpose)
TRANSPOSE_PATH = os.environ.get("KT_TRANSPOSE", "xbar")
LOAD_TILE_F = int(os.environ.get("KT_LOAD_F", "4096"))

_CACHE: dict = {}


def _build_program():
    nc = bacc.Bacc("TRN2", target_bir_lowering=False, debug=False,
                   num_devices=NCORES)

    x_d = nc.dram_tensor("x", [NROW, NPIX], F32, kind="ExternalInput").ap()
    # idx / wt chunk-transposed: [128, NCHUNK]; column c = values for chunk c
    idx_d = nc.dram_tensor("idxt", [PCHUNK, NCHUNK], F32,
                           kind="ExternalInput").ap()
    wt_d = nc.dram_tensor("wtt", [PCHUNK, NCHUNK], F32,
                          kind="ExternalInput").ap()
    iota_d = nc.dram_tensor("iota", [PCHUNK, RPAD], F16,
                            kind="ExternalInput").ap()
    out_d = nc.dram_tensor("out", [NROW, MAX_R], F32,
                           kind="ExternalOutput").ap()

    with tile.TileContext(nc) as tc, ExitStack() as ctx:
        const_pool = ctx.enter_context(tc.tile_pool(name="const", bufs=1))
        xin_pool = ctx.enter_context(tc.tile_pool(name="xin", bufs=3))
        x2_pool = ctx.enter_context(tc.tile_pool(name="x2", bufs=3))
        xt_pool = ctx.enter_context(tc.tile_pool(name="xt", bufs=3))
        oh_pool = ctx.enter_context(tc.tile_pool(name="oh", bufs=8))
        acc_pool = ctx.enter_context(
            tc.tile_pool(name="acc", bufs=1, space="PSUM"))

        idx_t = const_pool.tile([PCHUNK, NCHUNK], F32)
        nc.sync.dma_start(idx_t[:], idx_d[:])
        wt_t = const_pool.tile([PCHUNK, NCHUNK], F32)
        nc.sync.dma_start(wt_t[:], wt_d[:])
        iota_t = const_pool.tile([PCHUNK, RPAD], F16)
        nc.sync.dma_start(iota_t[:], iota_d[:])

        acc = acc_pool.tile([NROW, RPAD], F32)

        ntile = math.ceil(NPIX / LOAD_TILE_F)
        c = 0
        for t in range(ntile):
            f0 = t * LOAD_TILE_F
            fs = min(LOAD_TILE_F, NPIX - f0)
            nch = fs // PCHUNK
            xin = xin_pool.tile([NROW, LOAD_TILE_F], F32, tag="xin")
            nc.sync.dma_start(xin[:, :fs], x_d[:, f0:f0 + fs])
            x2 = x2_pool.tile([NROW, LOAD_TILE_F], F16, tag="x2")
            xt = xt_pool.tile([PCHUNK, LOAD_TILE_F // PCHUNK, NROW], F16,
                              tag="xt")
            # split each load tile in halves: square + slab-transpose of
            # half A overlap the DMA/compute of half B
            HALF = LOAD_TILE_F
            for h0 in range(0, fs, HALF):
                hs = min(HALF, fs - h0)
                # scale=32 -> values are 1024*x^2: keeps tiny x^2 out of
                # fp16 subnormals (undone by the 1/1024 in the final copy)
                nc.scalar.activation(x2[:, h0:h0 + hs], xin[:, h0:h0 + hs],
                                     mybir.ActivationFunctionType.Square,
                                     scale=32.0)
                # one xbar DMA slab-transposes hs//128 chunks:
                # xt[p, j, n] = x2[n, 128*j + p]
                j0 = h0 // PCHUNK
                nc.sync.dma_start_transpose(
                    xt[:, j0:j0 + hs // PCHUNK, :], x2[:, h0:h0 + hs])
                for j in range(j0, j0 + hs // PCHUNK):
                    oh = oh_pool.tile([PCHUNK, RPAD], F16, tag="oh")
                    eng = nc.vector if (c % 2 == 0) else nc.gpsimd
                    eng.tensor_scalar(
                        oh[:], iota_t[:],
                        scalar1=idx_t[:, c:c + 1], scalar2=wt_t[:, c:c + 1],
                        op0=mybir.AluOpType.is_equal,
                        op1=mybir.AluOpType.mult)
                    nc.tensor.matmul(acc[:], lhsT=xt[:, j, :], rhs=oh[:],
                                     start=(c == 0), stop=(c == NCHUNK - 1))
                    c += 1
        assert c == NCHUNK

        res = const_pool.tile([NROW, MAX_R], F32)
        nc.scalar.mul(res[:], acc[:, :MAX_R], 1.0 / 1024.0)
        nc.sync.dma_start(out_d[:], res[:])

    nc.compile()
    return nc


def _get_program():
    if "nc" not in _CACHE:
        _CACHE["nc"] = _build_program()
    return _CACHE["nc"]


def _host_prep(shell_index: np.ndarray, shells_weight: np.ndarray,
               shells_count: np.ndarray):
    idx_flat = shell_index.reshape(-1).astype(np.int64)
    wt = shells_weight.reshape(-1).astype(np.float64) / (
        shells_count.astype(np.float64)[idx_flat] + EPS)
    # chunk-transpose: A[i, c] = v[c*128 + i]
    idx_t = np.ascontiguousarray(
        idx_flat.reshape(NCHUNK, PCHUNK).T).astype(np.float32)
    wt_t = np.ascontiguousarray(
        wt.reshape(NCHUNK, PCHUNK).T).astype(np.float32)
    iota = np.broadcast_to(np.arange(RPAD, dtype=np.float16),
                           (PCHUNK, RPAD)).copy()
    return idx_t, wt_t, iota


def kernel(x: np.ndarray, shell_index: np.ndarray,
           shells_weight: np.ndarray, shells_count: np.ndarray,
           _trace: bool = False, **_tr_kwargs) -> np.ndarray:
    assert x.shape == (B, C, S, XDIM)
    nc = _get_program()
    idx_t, wt_t, iota = _host_prep(shell_index, shells_weight, shells_count)

    x = np.ascontiguousarray(x, dtype=np.float32)
    in_maps = []
    for k in range(NCORES):
        xk = x[k * BLOC:(k + 1) * BLOC].reshape(NROW, NPIX)
        in_maps.append({"x": xk, "idxt": idx_t, "wtt": wt_t, "iota": iota})

    res = run_bass_kernel_spmd(nc, in_maps, list(range(NCORES)),
                               trace=_trace, **_tr_kwargs)
    outs = [res.results[k]["out"] for k in range(NCORES)]
    full = np.concatenate(outs, axis=0).reshape(B, C, MAX_R).astype(np.float32)
    if _trace:
        return full, res
    return full



# revision 2
# speedup vs baseline: 1.9035x; 1.9035x over previous
"""Radial power-spectrum (GroupStat.get_spectrum) Trainium2 kernel.

Math:  out[b,c,r] = sum_{p: idx[p]==r} x[b,c,p]^2 * w[p] / (cnt[r]+eps)

Strategy (8 NeuronCores, data-parallel over batch B=128 -> 16 per core):
  * per core n = 16*8 = 128 rows (b_local, c) -> exactly the 128 SBUF
    partitions; pixels p = 256*129 = 33024 = 258 chunks of 128.
  * fold w[p]/(cnt[idx[p]]+eps) into a single per-pixel scalar wt[p] (host).
  * device pipeline per load tile (16 chunks):
      - DMA big fp32 tiles [128n, F] of x (natural layout) -- the only
        large DMA traffic; ~47us/core at HBM roofline.
      - ScalarE: square, cast -> fp16 (scale=32 -> 1024*x^2 keeps tiny
        values out of fp16 subnormals; undone in the final copy)
      - PE: transpose quads of [128n,128p] fp16 chunks -> psum [128p,4*128n]
        (keeps the DMA engines free of xbar-transpose traffic)
      - DVE: evict psum quad -> SBUF fp16
      - DVE/Pool: weighted one-hot [128p, 130r] = (iota == idx[p]) * wt[p]
        (single tensor_scalar, op0=is_equal, op1=mult; 3:2 DVE:Pool split)
      - PE: psum[128n, 130r] += xt.T @ onehot  (258 accumulating matmuls)
    PE work is software-pipelined one quad ahead of the matmuls so the
    engine never waits on the eviction.
  * psum -> SBUF -> DRAM [128, 129] per core; host stacks to [128,8,129].
"""

import os
from contextlib import ExitStack

import numpy as np

from concourse import bass, bacc, mybir
import concourse.tile as tile
from concourse.bass_utils import run_bass_kernel_spmd

B, C, S, XDIM = 128, 8, 256, 129
MAX_R = XDIM  # 129 shells
EPS = 1e-5
NCORES = 8
BLOC = B // NCORES          # 16 batches per core
NROW = BLOC * C             # 128 rows per core -> partition dim
NPIX = S * XDIM             # 33024 pixels
PCHUNK = 128
NCHUNK = NPIX // PCHUNK     # 258 (exact)
RPAD = 130                  # even free dim for DVE 4x mode; col 129 unused
QUAD = 4                    # chunks per psum-transpose/evict batch

F32 = mybir.dt.float32
F16 = mybir.dt.float16

LOAD_TILE_F = int(os.environ.get("KT_LOAD_F", "2048"))
assert LOAD_TILE_F % (PCHUNK * QUAD) == 0

_CACHE: dict = {}


def _build_program():
    nc = bacc.Bacc("TRN2", target_bir_lowering=False, debug=False,
                   num_devices=NCORES)

    x_d = nc.dram_tensor("x", [NROW, NPIX], F32, kind="ExternalInput").ap()
    # idx / wt chunk-transposed: [128, NCHUNK]; column c = values for chunk c
    idx_d = nc.dram_tensor("idxt", [PCHUNK, NCHUNK], F32,
                           kind="ExternalInput").ap()
    wt_d = nc.dram_tensor("wtt", [PCHUNK, NCHUNK], F32,
                          kind="ExternalInput").ap()
    iota_d = nc.dram_tensor("iota", [PCHUNK, RPAD], F16,
                            kind="ExternalInput").ap()
    ident_d = nc.dram_tensor("ident", [PCHUNK, PCHUNK], F16,
                             kind="ExternalInput").ap()
    out_d = nc.dram_tensor("out", [NROW, MAX_R], F32,
                           kind="ExternalOutput").ap()

    with tile.TileContext(nc) as tc, ExitStack() as ctx:
        const_pool = ctx.enter_context(tc.tile_pool(name="const", bufs=1))
        xin_pool = ctx.enter_context(tc.tile_pool(name="xin", bufs=3))
        x2_pool = ctx.enter_context(tc.tile_pool(name="x2", bufs=3))
        xt_pool = ctx.enter_context(tc.tile_pool(name="xt", bufs=3))
        oh_pool = ctx.enter_context(tc.tile_pool(name="oh", bufs=8))
        tp_pool = ctx.enter_context(
            tc.tile_pool(name="tp", bufs=3, space="PSUM"))
        acc_pool = ctx.enter_context(
            tc.tile_pool(name="acc", bufs=1, space="PSUM"))

        # first x tile before the (tiny) consts: the loads own DMA_ENGINES,
        # so front-load the big transfer the pipeline blocks on
        xin0 = xin_pool.tile([NROW, LOAD_TILE_F], F32, tag="xin")
        nc.sync.dma_start(xin0[:], x_d[:, 0:LOAD_TILE_F])

        idx_t = const_pool.tile([PCHUNK, NCHUNK], F32)
        nc.sync.dma_start(idx_t[:], idx_d[:])
        wt_t = const_pool.tile([PCHUNK, NCHUNK], F32)
        nc.sync.dma_start(wt_t[:], wt_d[:])
        iota_t = const_pool.tile([PCHUNK, RPAD], F16)
        nc.sync.dma_start(iota_t[:], iota_d[:])
        ident_t = const_pool.tile([PCHUNK, PCHUNK], F16)
        nc.sync.dma_start(ident_t[:], ident_d[:])

        acc = acc_pool.tile([NROW, RPAD], F32)

        def build_onehot(c):
            oh = oh_pool.tile([PCHUNK, RPAD], F16, tag="oh")
            eng = nc.vector if (c % 5 < 3) else nc.gpsimd
            eng.tensor_scalar(
                oh[:], iota_t[:],
                scalar1=idx_t[:, c:c + 1], scalar2=wt_t[:, c:c + 1],
                op0=mybir.AluOpType.is_equal,
                op1=mybir.AluOpType.mult)
            return oh

        # quads of transposed chunks flow through psum; matmuls run one quad
        # behind the transposes so PE never waits on the DVE eviction
        pending = None  # (xt_tile, first_chunk, nch)

        def flush_pending():
            nonlocal pending
            if pending is None:
                return
            xt, c0, nch = pending
            for j in range(nch):
                c = c0 + j
                oh = build_onehot(c)
                nc.tensor.matmul(acc[:], lhsT=xt[:, j * NROW:(j + 1) * NROW],
                                 rhs=oh[:],
                                 start=(c == 0), stop=(c == NCHUNK - 1))
            pending = None

        nload = (NPIX + LOAD_TILE_F - 1) // LOAD_TILE_F
        for t in range(nload):
            f0 = t * LOAD_TILE_F
            fs = min(LOAD_TILE_F, NPIX - f0)
            if t == 0:
                xin = xin0
            else:
                xin = xin_pool.tile([NROW, LOAD_TILE_F], F32, tag="xin")
                nc.sync.dma_start(xin[:, :fs], x_d[:, f0:f0 + fs])
            x2 = x2_pool.tile([NROW, LOAD_TILE_F], F16, tag="x2")
            # scale=32 -> values are 1024*x^2: keeps tiny x^2 out of
            # fp16 subnormals (undone by the 1/1024 in the final copy)
            nc.scalar.activation(x2[:, :fs], xin[:, :fs],
                                 mybir.ActivationFunctionType.Square,
                                 scale=32.0)
            for q0 in range(0, fs // PCHUNK, QUAD):
                nch = min(QUAD, fs // PCHUNK - q0)
                tp = tp_pool.tile([PCHUNK, QUAD * NROW], F16, tag="tp")
                for j in range(nch):
                    h0 = (q0 + j) * PCHUNK
                    nc.tensor.transpose(
                        tp[:, j * NROW:(j + 1) * NROW],
                        x2[:, h0:h0 + PCHUNK], ident_t[:])
                xt = xt_pool.tile([PCHUNK, QUAD * NROW], F16, tag="xt")
                nc.vector.tensor_copy(xt[:, :nch * NROW],
                                      tp[:, :nch * NROW])
                flush_pending()
                pending = (xt, f0 // PCHUNK + q0, nch)
        flush_pending()

        res = const_pool.tile([NROW, MAX_R], F32)
        nc.scalar.mul(res[:], acc[:, :MAX_R], 1.0 / 1024.0)
        nc.sync.dma_start(out_d[:], res[:])

    nc.compile()
    return nc


def _get_program():
    if "nc" not in _CACHE:
        _CACHE["nc"] = _build_program()
    return _CACHE["nc"]


def _host_prep(shell_index: np.ndarray, shells_weight: np.ndarray,
               shells_count: np.ndarray):
    idx_flat = shell_index.reshape(-1).astype(np.int64)
    wt = shells_weight.reshape(-1).astype(np.float64) / (
        shells_count.astype(np.float64)[idx_flat] + EPS)
    # chunk-transpose: A[i, c] = v[c*128 + i]
    idx_t = np.ascontiguousarray(
        idx_flat.reshape(NCHUNK, PCHUNK).T).astype(np.float32)
    wt_t = np.ascontiguousarray(
        wt.reshape(NCHUNK, PCHUNK).T).astype(np.float32)
    iota = np.broadcast_to(np.arange(RPAD, dtype=np.float16),
                           (PCHUNK, RPAD)).copy()
    ident = np.eye(PCHUNK, dtype=np.float16)
    return idx_t, wt_t, iota, ident


def kernel(x: np.ndarray, shell_index: np.ndarray,
           shells_weight: np.ndarray, shells_count: np.ndarray,
           _trace: bool = False, **_tr_kwargs) -> np.ndarray:
    assert x.shape == (B, C, S, XDIM)
    nc = _get_program()
    idx_t, wt_t, iota, ident = _host_prep(shell_index, shells_weight,
                                          shells_count)

    x = np.ascontiguousarray(x, dtype=np.float32)
    in_maps = []
    for k in range(NCORES):
        xk = x[k * BLOC:(k + 1) * BLOC].reshape(NROW, NPIX)
        in_maps.append({"x": xk, "idxt": idx_t, "wtt": wt_t, "iota": iota,
                        "ident": ident})

    res = run_bass_kernel_spmd(nc, in_maps, list(range(NCORES)),
                               trace=_trace, **_tr_kwargs)
    outs = [res.results[k]["out"] for k in range(NCORES)]
    full = np.concatenate(outs, axis=0).reshape(B, C, MAX_R).astype(np.float32)
    if _trace:
        return full, res
    return full
